# revision 21
# baseline (speedup 1.0000x reference)
"""AttentionEncoder TRN2 Bass kernel (v2: fp8 DoubleRow attention + bf16 pipeline).

Data-parallel over batch: B=8 samples -> 8 NeuronCores, one sample per core.

Math identity used for the attention matmul (the dominant 4.3 GMAC):
  scores = sigmoid(z), z = qk + bias
  attention = scores @ v = tanh(z/2) @ (v/2) + colsum(v/2)
The tanh term runs as fp8e4 DoubleRow matmuls (2x MACs/instr); centering via
tanh makes the fp8 quantization of scores ~3x finer, and computing the
colsum correction exactly (colsum(v/2) = xsum @ (Wv/2) + S*bv/2, xsum from a
DVE free-dim reduce of x^T) cancels the mean-term of the fp8 value
quantization error. z/2 is produced directly by folding 0.5 into Wk/Wb on
host. All other matmuls run in bf16 (same PE rate as fp32r but weight loads
fully hidden); res stays resident in SBUF as bf16 (no DRAM spill).

Per-core phases (S=2048, D=1024, K=64):
  phase 0: kq^T = [Wk*s/2|Wq]^T@x^T (+bias row trick), colsum via xsum@Wvh,
           vh = x@(Wv/2) s-major -> fp8e4
  phase 1: per 512-col superblock: z/2 matmuls -> tanh -> fp8 strips;
           attention += strips(DR pairs)@vh; epilogue rp=pa+(x+colsumB),
           LN1 via bn_stats -> res_bf16 resident
  phase 2: per 128-row chunk: PE-transpose res, FF matmul bf16,
           relu+residual, LN2, DMA out.
"""
import numpy as np
import ml_dtypes
from contextlib import ExitStack

import concourse.bass as bass
import concourse.tile as tile
from concourse import bacc, mybir
from concourse.bass_utils import run_bass_kernel_spmd
from concourse.alu_op_type import AluOpType

F32 = mybir.dt.float32
BF16 = mybir.dt.bfloat16
F8E4 = mybir.dt.float8e4
ACTF = mybir.ActivationFunctionType
DRMODE = mybir.MatmulPerfMode.DoubleRow

B, S, D, K = 8, 2048, 1024, 64
EPS = 1e-5
NCORES = 8
SB = 512          # superblock width (scores free dim)
NSB = S // SB     # 4
NC = S // 128     # 16 s-chunks
ND2 = D // 512    # 2 d-tiles


def build_program(flags):
    have_bkq, have_bb, have_bv, have_b1, have_gb = flags
    nc = bacc.Bacc(trn_type="TRN2")

    xb_d = nc.declare_dram_parameter("xb", [128, 8, S], BF16, isOutput=False)
    x_d = nc.declare_dram_parameter("x", [S, D], F32, isOutput=False)
    wkq_d = nc.declare_dram_parameter("wkq", [128, 8, 128], BF16, isOutput=False)
    wb_d = nc.declare_dram_parameter("wb", [128, 8, 1], BF16, isOutput=False)
    wv_d = nc.declare_dram_parameter("wv", [128, 8, D], BF16, isOutput=False)
    w1_d = nc.declare_dram_parameter("w1", [128, 8, D], BF16, isOutput=False)
    ones_d = nc.declare_dram_parameter("onesrow", [1, S], BF16, isOutput=False)
    iden_d = nc.declare_dram_parameter("iden", [128, 128], BF16, isOutput=False)
    g1_d = nc.declare_dram_parameter("g1", [1, D], F32, isOutput=False)
    be1_d = nc.declare_dram_parameter("be1", [1, D], F32, isOutput=False)
    bkq_d = nc.declare_dram_parameter("bkq", [1, 128], BF16, isOutput=False)
    bb_d = nc.declare_dram_parameter("bb", [1, 1], BF16, isOutput=False)
    bvs_d = nc.declare_dram_parameter("bvs", [1, D], BF16, isOutput=False)
    bvcs_d = nc.declare_dram_parameter("bvcs", [1, D], BF16, isOutput=False)
    b1_d = nc.declare_dram_parameter("b1", [1, D], BF16, isOutput=False)
    out_d = nc.declare_dram_parameter("out", [S, D], F32, isOutput=True)

    m_spill = nc.dram_tensor("m_spill", [1, S], BF16)

    with tile.TileContext(nc) as tc, ExitStack() as top:
        const = top.enter_context(tc.tile_pool(name="const", bufs=1))
        kqp = top.enter_context(tc.tile_pool(name="kqp", bufs=1))
        vp = top.enter_context(tc.tile_pool(name="vp", bufs=1))
        resp = top.enter_context(tc.tile_pool(name="resp", bufs=1))
        csp = top.enter_context(tc.tile_pool(name="csp", bufs=1))

        # ---- constants
        eps_t = const.tile([128, 1], F32)
        nc.vector.memset(eps_t, EPS)
        zero_t = const.tile([128, 1], F32)
        nc.vector.memset(zero_t, 0.0)
        magic_t = const.tile([128, 1], mybir.dt.int32)
        nc.vector.memset(magic_t, 0x5f3759df)
        one_i = const.tile([128, 1], mybir.dt.int32)
        nc.vector.memset(one_i, 1)
        neghalf_t = const.tile([128, 1], F32)
        nc.vector.memset(neghalf_t, -0.5)
        threehalf_t = const.tile([128, 1], F32)
        nc.vector.memset(threehalf_t, 1.5)
        negone_t = const.tile([128, 1], F32)
        nc.vector.memset(negone_t, -1.0)
        I32 = mybir.dt.int32

        def emit_rstd(pool, var_ap, nm):
            """1/sqrt(var+EPS) on DVE only (bit trick + 2 Newton iters)."""
            scr = pool.tile([128, 5], F32, tag="scr", name=f"scr{nm}")
            vpe, y = scr[:, 0:1], scr[:, 1:2]
            y2, b = scr[:, 2:3], scr[:, 3:4]
            d = scr[:, 4:5]
            nc.vector.tensor_tensor(vpe, var_ap, eps_t, op=AluOpType.add)
            nc.vector.tensor_scalar(
                out=y.bitcast(I32), in0=vpe.bitcast(I32), scalar1=one_i,
                scalar2=None, op0=AluOpType.logical_shift_right)
            nc.vector.tensor_tensor(out=y.bitcast(I32), in0=magic_t,
                                    in1=y.bitcast(I32), op=AluOpType.subtract)
            for _ in range(2):
                nc.vector.tensor_tensor(y2, y, y, op=AluOpType.mult)
                nc.vector.tensor_tensor(b, vpe, y2, op=AluOpType.mult)
                nc.vector.scalar_tensor_tensor(
                    out=d, in0=b, scalar=neghalf_t, in1=threehalf_t,
                    op0=AluOpType.mult, op1=AluOpType.add)
                nc.vector.tensor_tensor(y, y, d, op=AluOpType.mult)
            return y

        ones_t = const.tile([1, S], BF16)
        nc.gpsimd.dma_start(ones_t, ones_d.ap())
        iden_t = const.tile([128, 128], BF16)
        nc.gpsimd.dma_start(iden_t, iden_d.ap())
        if have_gb:
            g1_b = const.tile([128, D], F32)
            nc.sync.dma_start(g1_b, g1_d.ap().partition_broadcast(128))
            be1_b = const.tile([128, D], F32)
            nc.sync.dma_start(be1_b, be1_d.ap().partition_broadcast(128))
        if have_bkq:
            bkq_t = const.tile([1, 128], BF16)
            nc.sync.dma_start(bkq_t, bkq_d.ap())
        if have_bb:
            bb_t = const.tile([1, 1], BF16)
            nc.sync.dma_start(bb_t, bb_d.ap())
        if have_bv:
            bvs_t = const.tile([1, D], BF16)
            nc.sync.dma_start(bvs_t, bvs_d.ap())
            bvcs_t = const.tile([1, D], BF16)
            nc.sync.dma_start(bvcs_t, bvcs_d.ap())
        if have_b1:
            b1_t = const.tile([1, D], BF16)
            nc.sync.dma_start(b1_t, b1_d.ap())

        # ---- kq/bias output tiles (rows 0..64)
        tile_k = kqp.tile([65, S], BF16)   # rows0-63 keysT*(s/2), row64 biasT/2
        tile_q = kqp.tile([65, S], BF16)   # rows0-63 queriesT, row64 ones
        nc.gpsimd.dma_start(tile_q[64:65, :], ones_d.ap())

        # ---- vh: values/2 s-major fp8, resident through phase 1
        vh_sb = vp.tile([128, NC, D], F8E4)
        # ---- res: LN1 output, bf16, resident through phase 2
        res_b = resp.tile([128, NC, D], BF16)
        # ---- per-row score centers and colsum correction row
        mB = csp.tile([128, S], BF16)        # broadcast of m = tanh(bias/2)
        onepm = csp.tile([1, S], BF16)       # 1 + m
        csrow = csp.tile([1, D], BF16)       # colsum(vh) exact
        one_f = const.tile([1, 1], F32)
        nc.vector.memset(one_f, 1.0)

        # ================= phase 0: projections =================
        with ExitStack() as ph0:
            xp = ph0.enter_context(tc.tile_pool(name="xp", bufs=1))
            wp = ph0.enter_context(tc.tile_pool(name="wp", bufs=1))

            # weights first (small, needed immediately), then x^T in
            # half-chunks ordered to match kq consumption (h=0 first)
            wkq_t = xp.tile([128, 8, 128], BF16)
            nc.sync.dma_start(wkq_t, wkq_d.ap())
            wb_t = xp.tile([128, 8, 1], BF16)
            nc.scalar.dma_start(wb_t, wb_d.ap())
            xt_t = xp.tile([128, 8, S], BF16)
            qs = [nc.sync, nc.scalar, nc.gpsimd]
            for h in range(2):
                for k in range(8):
                    sl = slice(h * 1024, (h + 1) * 1024)
                    qs[k % 3].dma_start(xt_t[:, k, sl], xb_d[:, k, sl])
            wv_t = wp.tile([128, 8, D], BF16, tag="wv")
            nc.gpsimd.dma_start(wv_t[:, 0:2, :], wv_d[:, 0:2, :])
            nc.gpsimd.dma_start(wv_t[:, 2:4, :], wv_d[:, 2:4, :])
            nc.gpsimd.dma_start(wv_t[:, 4:6, :], wv_d[:, 4:6, :])
            nc.gpsimd.dma_start(wv_t[:, 6:8, :], wv_d[:, 6:8, :])

            # xsum[d] = sum_s xT[d, s]  (for the exact colsum correction),
            # split per chunk so each runs as soon as its DMA lands
            xsum_f = xp.tile([128, 8], F32, tag="xsf")
            for k in range(8):
                nc.vector.tensor_reduce(xsum_f[:, k:k + 1], xt_t[:, k, :],
                                        axis=mybir.AxisListType.X,
                                        op=AluOpType.add)
            xsum_b = xp.tile([128, 8], BF16, tag="xsb")
            nc.vector.tensor_copy(xsum_b, xsum_f)

            # kq + bias, k-outer in two passes (PSUM: pkq 2 + pb 2 banks)
            ph0kq = ph0.enter_context(ExitStack())
            pkq = ph0kq.enter_context(
                tc.tile_pool(name="pkq", bufs=2, space="PSUM"))
            pb = ph0kq.enter_context(
                tc.tile_pool(name="pb", bufs=2, space="PSUM"))
            for half in range(2):
                sts = (0, 1) if half == 0 else (2, 3)
                pks = {st: pkq.tile([128, 512], F32, tag="kq",
                                    name=f"pk{st}") for st in sts}
                pbs = {st: pb.tile([1, 512], F32, tag="b",
                                   name=f"pb{st}") for st in sts}
                for k in range(8):
                    for st in sts:   # consecutive pair shares stationary
                        sl = slice(st * SB, (st + 1) * SB)
                        nc.tensor.matmul(pks[st], wkq_t[:, k, :],
                                         xt_t[:, k, sl], start=(k == 0),
                                         stop=(k == 7 and not have_bkq))
                for k in range(8):
                    for st in sts:
                        sl = slice(st * SB, (st + 1) * SB)
                        nc.tensor.matmul(pbs[st], wb_t[:, k, :],
                                         xt_t[:, k, sl], start=(k == 0),
                                         stop=(k == 7 and not have_bb))
                for st in sts:
                    sl = slice(st * SB, (st + 1) * SB)
                    if have_bkq:
                        nc.tensor.matmul(pks[st], bkq_t, ones_t[:, sl],
                                         start=False, stop=True)
                    if have_bb:
                        nc.tensor.matmul(pbs[st], bb_t, ones_t[:, sl],
                                         start=False, stop=True)
                    nc.scalar.activation(tile_k[0:64, sl], pks[st][0:64, :],
                                         ACTF.Copy)
                    nc.vector.tensor_copy(tile_q[0:64, sl],
                                          pks[st][64:128, :])
                    nc.scalar.activation(tile_k[64:65, sl], pbs[st], ACTF.Copy)

            # per-row centers m = tanh(biasT/2); broadcast via DRAM roundtrip
            m_row = xp.tile([1, S], BF16, tag="mrow")
            nc.scalar.activation(m_row, tile_k[64:65, :], ACTF.Tanh)
            nc.vector.tensor_scalar(out=onepm, in0=m_row, scalar1=one_f,
                                    scalar2=None, op0=AluOpType.add)
            nc.sync.dma_start(m_spill.ap(), m_row)
            nc.sync.dma_start(mB, m_spill.ap().partition_broadcast(128))

            # colsum(vh) = xsum @ (Wv/2) (+ S*bv/2), kept as bf16 row
            ph0kq.close()
            pcs = ph0.enter_context(
                tc.tile_pool(name="pcs", bufs=2, space="PSUM"))
            pcst = {dt2: pcs.tile([1, 512], F32, tag="cs", name=f"cs{dt2}")
                    for dt2 in range(ND2)}
            for k in range(8):
                for dt2 in range(ND2):
                    dsl = slice(dt2 * 512, (dt2 + 1) * 512)
                    nc.tensor.matmul(pcst[dt2], xsum_b[:, k:k + 1],
                                     wv_t[:, k, dsl], start=(k == 0),
                                     stop=(k == 7 and not have_bv))
            if have_bv:
                for dt2 in range(ND2):
                    dsl = slice(dt2 * 512, (dt2 + 1) * 512)
                    nc.tensor.matmul(pcst[dt2], ones_t[0:1, 0:1],
                                     bvcs_t[:, dsl], start=False, stop=True)
            for dt2 in range(ND2):
                nc.scalar.activation(csrow[:, dt2 * 512:(dt2 + 1) * 512],
                                     pcst[dt2], ACTF.Copy)

            # values/2: out[s-chunk,128 x d-512] -> fp8, stationary xT reused
            pv = ph0.enter_context(
                tc.tile_pool(name="pv", bufs=4, space="PSUM"))
            for sc in range(NC):
                pvt = {dt2: pv.tile([128, 512], F32, tag="v", name=f"pv{dt2}")
                       for dt2 in range(ND2)}
                for k in range(8):
                    for dt2 in range(ND2):   # pair shares stationary
                        nc.tensor.matmul(
                            pvt[dt2], xt_t[:, k, sc * 128:(sc + 1) * 128],
                            wv_t[:, k, dt2 * 512:(dt2 + 1) * 512],
                            start=(k == 0), stop=(k == 7 and not have_bv))
                if have_bv:
                    for dt2 in range(ND2):
                        nc.tensor.matmul(
                            pvt[dt2], ones_t[:, 0:128],
                            bvs_t[:, dt2 * 512:(dt2 + 1) * 512],
                            start=False, stop=True)
                for dt2 in range(ND2):
                    nc.scalar.activation(
                        vh_sb[:, sc, dt2 * 512:(dt2 + 1) * 512], pvt[dt2],
                        ACTF.Copy)

        # ================= phase 1: scores + attention + LN1 =================
        w1p = top.enter_context(tc.tile_pool(name="w1p", bufs=1))
        w1_t = w1p.tile([128, 8, D], BF16)
        nc.scalar.dma_start(w1_t[:, 0:4, :], w1_d[:, 0:4, :])
        nc.gpsimd.dma_start(w1_t[:, 4:8, :], w1_d[:, 4:8, :])

        with ExitStack() as ph1i:
            strips = ph1i.enter_context(tc.tile_pool(name="strips", bufs=4))
            tpool = ph1i.enter_context(tc.tile_pool(name="tpool", bufs=3))
            xin = ph1i.enter_context(tc.tile_pool(name="xin", bufs=3))
            rpre = ph1i.enter_context(tc.tile_pool(name="rpre", bufs=3))
            stat = ph1i.enter_context(tc.tile_pool(name="stat", bufs=6))
            ps_s = ph1i.enter_context(
                tc.tile_pool(name="ps_s", bufs=4, space="PSUM"))
            ps_a = ph1i.enter_context(
                tc.tile_pool(name="ps_a", bufs=4, space="PSUM"))

            for sb in range(NSB):
                isl = slice(sb * SB, (sb + 1) * SB)
                halves = [strips.tile([128, 4, SB], F8E4, tag="strip",
                                      name=f"strip_{sb}_{q}")
                          for q in range(4)]
                for j in range(NC):
                    pst = ps_s.tile([128, SB], F32, tag="s")
                    nc.tensor.matmul(
                        pst, tile_q[:, j * 128:(j + 1) * 128],
                        tile_k[:, isl], start=True, stop=True)
                    tt = tpool.tile([128, SB], F32, tag="tt",
                                    name=f"tt{j % 3}")
                    nc.scalar.activation(tt, pst, ACTF.Tanh)
                    # center per row i: t' = tanh(z/2) - m_i  (fp8 cast);
                    # alternate DVE / GpSimd so strip production keeps up
                    eng = nc.vector if j % 2 == 0 else nc.gpsimd
                    eng.tensor_tensor(
                        halves[j // 4][:, j % 4, :], tt, mB[:, isl],
                        op=AluOpType.subtract)

                for l in range(4):
                    c = sb * 4 + l   # global 128-row chunk
                    x_t = xin.tile([128, D], F32, tag="x", name=f"x{l}")
                    nc.gpsimd.dma_start(x_t, x_d[c * 128:(c + 1) * 128, :])
                    pa = [ps_a.tile([128, 512], F32, tag="a", name=f"pa{_d}")
                          for _d in range(ND2)]
                    csl = slice(c * 128, (c + 1) * 128)
                    for dt2 in range(ND2):
                        # rank-1 seed: (1+m_i) * colsum(vh)[d]
                        nc.tensor.matmul(
                            pa[dt2], onepm[0:1, csl],
                            csrow[0:1, dt2 * 512:(dt2 + 1) * 512],
                            start=True, stop=False)
                    for jp in range(8):      # DoubleRow over j-chunk pairs
                        q, m = jp // 2, (jp % 2) * 2
                        st_ap = halves[q][:, m:m + 2, l * 128:(l + 1) * 128]
                        for dt2 in range(ND2):   # pair shares stationary
                            nc.tensor.matmul(
                                pa[dt2], st_ap,
                                vh_sb[:, 2 * jp:2 * jp + 2,
                                      dt2 * 512:(dt2 + 1) * 512],
                                start=False, stop=(jp == 7),
                                perf_mode=DRMODE)
                    rp = rpre.tile([128, D], F32, tag="rp", name=f"rp{l}")
                    for dt2 in range(ND2):
                        dsl = slice(dt2 * 512, (dt2 + 1) * 512)
                        nc.vector.tensor_tensor(
                            rp[:, dsl], pa[dt2], x_t[:, dsl],
                            op=AluOpType.add)
                    st_t = stat.tile([128, 2, 6], F32, tag="bst",
                                     name=f"bst{l}")
                    for g in range(2):
                        nc.vector.bn_stats(st_t[:, g, :],
                                           rp[:, g * 512:(g + 1) * 512])
                    mv = stat.tile([128, 2], F32, tag="mv", name=f"mv{l}")
                    nc.vector.bn_aggr(mv, st_t)
                    rstd = emit_rstd(stat, mv[:, 1:2], f"r1_{c}")
                    if have_gb:
                        t1 = rpre.tile([128, D], F32, tag="t1", name=f"t1{l}")
                        nc.vector.scalar_tensor_tensor(
                            out=t1, in0=rp, scalar=mv[:, 0:1],
                            in1=g1_b,
                            op0=AluOpType.subtract, op1=AluOpType.mult)
                        nc.vector.scalar_tensor_tensor(
                            out=res_b[:, c, :], in0=t1, scalar=rstd,
                            in1=be1_b,
                            op0=AluOpType.mult, op1=AluOpType.add)
                    else:
                        # normalize on scalar: rp*rstd + (-mean*rstd)
                        nmr = stat.tile([128, 1], F32, tag="nmr",
                                        name=f"nmr{l}")
                        nc.vector.tensor_scalar(
                            out=nmr, in0=mv[:, 0:1], scalar1=rstd,
                            scalar2=negone_t, op0=AluOpType.mult,
                            op1=AluOpType.mult)
                        nc.scalar.activation(res_b[:, c, :], rp,
                                             ACTF.Identity, bias=nmr,
                                             scale=rstd)

        # ================= phase 2: FF + LN2 =================
        with ExitStack() as ph2:
            rtp = ph2.enter_context(tc.tile_pool(name="rtp", bufs=4))
            f2 = ph2.enter_context(tc.tile_pool(name="f2", bufs=2))
            ostage = ph2.enter_context(tc.tile_pool(name="ostage", bufs=2))
            stat2 = ph2.enter_context(tc.tile_pool(name="stat2", bufs=6))
            ps_t = ph2.enter_context(
                tc.tile_pool(name="ps_t", bufs=4, space="PSUM"))
            ps_f = ph2.enter_context(
                tc.tile_pool(name="ps_f", bufs=4, space="PSUM"))

            rT = [None] * NC

            def stage_tr(c):
                rt_ = rtp.tile([128, 8, 128], BF16, tag="rT")
                for h in range(2):
                    ptr = ps_t.tile([128, 512], BF16, tag="tr",
                                    name=f"ptr{h}")
                    for k in range(4):
                        kk = h * 4 + k
                        nc.tensor.transpose(
                            ptr[:, k * 128:(k + 1) * 128],
                            res_b[:, c, kk * 128:(kk + 1) * 128], iden_t)
                    nc.scalar.activation(rt_[:, h * 4:(h + 1) * 4, :], ptr,
                                         ACTF.Copy)
                rT[c] = rt_

            def stage_ff(c):
                pf = {dt2: ps_f.tile([128, 512], F32, tag="f", name=f"pf{_d}")
                      for dt2, _d in ((d_, d_) for d_ in range(ND2))}
                for k in range(8):
                    for dt2 in range(ND2):   # pair shares stationary
                        dsl = slice(dt2 * 512, (dt2 + 1) * 512)
                        nc.tensor.matmul(pf[dt2], rT[c][:, k, :],
                                         w1_t[:, k, dsl],
                                         start=(k == 0),
                                         stop=(k == 7 and not have_b1))
                if have_b1:
                    for dt2 in range(ND2):
                        dsl = slice(dt2 * 512, (dt2 + 1) * 512)
                        nc.tensor.matmul(pf[dt2], ones_t[0:1, 0:128],
                                         b1_t[:, dsl], start=False, stop=True)
                r2 = f2.tile([128, D], F32, tag="r2")
                for dt2 in range(ND2):
                    dsl = slice(dt2 * 512, (dt2 + 1) * 512)
                    nc.vector.scalar_tensor_tensor(
                        out=r2[:, dsl], in0=pf[dt2], scalar=zero_t,
                        in1=res_b[:, c, dsl],
                        op0=AluOpType.max, op1=AluOpType.add)
                st_t = stat2.tile([128, 2, 6], F32, tag="bst2")
                for g in range(2):
                    nc.vector.bn_stats(st_t[:, g, :],
                                       r2[:, g * 512:(g + 1) * 512])
                mv = stat2.tile([128, 2], F32, tag="mv2")
                nc.vector.bn_aggr(mv, st_t)
                rstd = emit_rstd(stat2, mv[:, 1:2], f"r2_{c}")
                o_t = ostage.tile([128, D], F32, tag="o")
                nmr = stat2.tile([128, 1], F32, tag="nmr2")
                nc.vector.tensor_scalar(
                    out=nmr, in0=mv[:, 0:1], scalar1=rstd,
                    scalar2=negone_t, op0=AluOpType.mult,
                    op1=AluOpType.mult)
                nc.scalar.activation(o_t, r2, ACTF.Identity, bias=nmr,
                                     scale=rstd)
                nc.sync.dma_start(out_d[c * 128:(c + 1) * 128, :], o_t)

            for c in range(NC + 1):
                if c < NC:
                    stage_tr(c)
                if c >= 1:
                    stage_ff(c - 1)

    nc.finalize()
    return nc


_PROGRAM_CACHE = {}


def _get_program(flags):
    if flags not in _PROGRAM_CACHE:
        _PROGRAM_CACHE[flags] = build_program(flags)
    return _PROGRAM_CACHE[flags]


def kernel(x, Wk, bk, Wq, bq, Wv, bv, Wb, bb, W1, b1, g1, be1):
    import math
    scale = 1.0 / math.sqrt(K)
    flags = (
        bool(np.any(bk) or np.any(bq)),
        bool(np.any(bb)),
        bool(np.any(bv)),
        bool(np.any(b1)),
        bool(np.any(g1 != 1.0) or np.any(be1)),
    )
    nc = _get_program(flags)

    def _prep(w):
        # [D, M] -> [128, 8, M] matching SBUF [partition, chunk, free], bf16
        return np.ascontiguousarray(
            w.astype(np.float32).reshape(8, 128, -1).transpose(1, 0, 2)
        ).astype(ml_dtypes.bfloat16)

    # z/2 trick: keys side gets scale/2, bias side gets 1/2
    wkq = _prep(np.concatenate([Wk * (scale * 0.5), Wq], axis=1))
    wb = _prep(np.asarray(Wb) * 0.5)
    wv = _prep(np.asarray(Wv) * 0.5)
    w1 = _prep(np.asarray(W1))
    onesrow = np.ones((1, S), dtype=ml_dtypes.bfloat16)
    iden = np.eye(128, dtype=ml_dtypes.bfloat16)
    bkq = np.concatenate([np.asarray(bk) * (scale * 0.5), np.asarray(bq)]
                         )[None, :].astype(ml_dtypes.bfloat16)
    bbr = (np.asarray(bb, dtype=np.float32) * 0.5).reshape(1, 1).astype(
        ml_dtypes.bfloat16)
    bvsr = (np.asarray(bv, dtype=np.float32) * 0.5)[None, :].astype(
        ml_dtypes.bfloat16)
    bvcsr = (np.asarray(bv, dtype=np.float32) * (0.5 * S))[None, :].astype(
        ml_dtypes.bfloat16)
    b1r = np.asarray(b1, dtype=np.float32)[None, :].astype(ml_dtypes.bfloat16)
    g1r = np.asarray(g1, dtype=np.float32)[None, :]
    be1r = np.asarray(be1, dtype=np.float32)[None, :]

    xb = np.ascontiguousarray(np.transpose(x, (0, 2, 1)))        # [B,D,S]
    xb = np.ascontiguousarray(
        xb.reshape(B, 8, 128, S).transpose(0, 2, 1, 3)            # [B,128,8,S]
    ).astype(ml_dtypes.bfloat16)

    in_maps = []
    for b in range(B):
        in_maps.append(dict(
            xb=xb[b], x=np.ascontiguousarray(x[b], dtype=np.float32),
            wkq=wkq, wb=wb, wv=wv, w1=w1, onesrow=onesrow, iden=iden,
            g1=g1r, be1=be1r, bkq=bkq, bb=bbr, bvs=bvsr, bvcs=bvcsr,
            b1=b1r))

    res = run_bass_kernel_spmd(nc, in_maps, list(range(NCORES)), trace=False)
    out = np.stack([res.results[b]["out"] for b in range(B)], axis=0)
    return out.astype(np.float32)


# revision 26
# speedup vs baseline: 1.0894x; 1.0894x over previous
"""AttentionEncoder TRN2 Bass kernel (v2: fp8 DoubleRow attention + bf16 pipeline).

Data-parallel over batch: B=8 samples -> 8 NeuronCores, one sample per core.

Math identity used for the attention matmul (the dominant 4.3 GMAC):
  scores = sigmoid(z), z = qk + bias
  attention = scores @ v = tanh(z/2) @ (v/2) + colsum(v/2)
The tanh term runs as fp8e4 DoubleRow matmuls (2x MACs/instr); centering via
tanh makes the fp8 quantization of scores ~3x finer, and computing the
colsum correction exactly (colsum(v/2) = xsum @ (Wv/2) + S*bv/2, xsum from a
DVE free-dim reduce of x^T) cancels the mean-term of the fp8 value
quantization error. z/2 is produced directly by folding 0.5 into Wk/Wb on
host. All other matmuls run in bf16 (same PE rate as fp32r but weight loads
fully hidden); res stays resident in SBUF as bf16 (no DRAM spill).

Per-core phases (S=2048, D=1024, K=64):
  phase 0: kq^T = [Wk*s/2|Wq]^T@x^T (+bias row trick), colsum via xsum@Wvh,
           vh = x@(Wv/2) s-major -> fp8e4
  phase 1: per 512-col superblock: z/2 matmuls -> tanh -> fp8 strips;
           attention += strips(DR pairs)@vh; epilogue rp=pa+(x+colsumB),
           LN1 via bn_stats -> res_bf16 resident
  phase 2: per 128-row chunk: PE-transpose res, FF matmul bf16,
           relu+residual, LN2, DMA out.
"""
import numpy as np
import ml_dtypes
from contextlib import ExitStack

import concourse.bass as bass
import concourse.tile as tile
from concourse import bacc, mybir
from concourse.bass_utils import run_bass_kernel_spmd
from concourse.alu_op_type import AluOpType

F32 = mybir.dt.float32
BF16 = mybir.dt.bfloat16
F8E4 = mybir.dt.float8e4
ACTF = mybir.ActivationFunctionType
DRMODE = mybir.MatmulPerfMode.DoubleRow

B, S, D, K = 8, 2048, 1024, 64
EPS = 1e-5
NCORES = 8
SB = 512          # superblock width (scores free dim)
NSB = S // SB     # 4
NC = S // 128     # 16 s-chunks
ND2 = D // 512    # 2 d-tiles


def build_program(flags):
    have_bkq, have_bb, have_bv, have_b1, have_gb = flags
    nc = bacc.Bacc(trn_type="TRN2")

    xb_d = nc.declare_dram_parameter("xb", [128, 8, S], BF16, isOutput=False)
    x_d = nc.declare_dram_parameter("x", [S, D], F32, isOutput=False)
    wkq_d = nc.declare_dram_parameter("wkq", [128, 8, 128], BF16, isOutput=False)
    wb_d = nc.declare_dram_parameter("wb", [128, 8, 1], BF16, isOutput=False)
    wv_d = nc.declare_dram_parameter("wv", [128, 8, D], BF16, isOutput=False)
    w1_d = nc.declare_dram_parameter("w1", [128, 8, D], BF16, isOutput=False)
    ones_d = nc.declare_dram_parameter("onesrow", [1, S], BF16, isOutput=False)
    iden_d = nc.declare_dram_parameter("iden", [128, 128], BF16, isOutput=False)
    g1_d = nc.declare_dram_parameter("g1", [1, D], F32, isOutput=False)
    be1_d = nc.declare_dram_parameter("be1", [1, D], F32, isOutput=False)
    bkq_d = nc.declare_dram_parameter("bkq", [1, 128], BF16, isOutput=False)
    bb_d = nc.declare_dram_parameter("bb", [1, 1], BF16, isOutput=False)
    bvs_d = nc.declare_dram_parameter("bvs", [1, D], BF16, isOutput=False)
    bvcs_d = nc.declare_dram_parameter("bvcs", [1, D], BF16, isOutput=False)
    b1_d = nc.declare_dram_parameter("b1", [1, D], BF16, isOutput=False)
    out_d = nc.declare_dram_parameter("out", [S, D], F32, isOutput=True)

    m_spill = nc.dram_tensor("m_spill", [1, S], BF16)

    with tile.TileContext(nc) as tc, ExitStack() as top:
        const = top.enter_context(tc.tile_pool(name="const", bufs=1))
        kqp = top.enter_context(tc.tile_pool(name="kqp", bufs=1))
        vp = top.enter_context(tc.tile_pool(name="vp", bufs=1))
        resp = top.enter_context(tc.tile_pool(name="resp", bufs=1))
        csp = top.enter_context(tc.tile_pool(name="csp", bufs=1))

        # ---- constants
        eps_t = const.tile([128, 1], F32)
        nc.vector.memset(eps_t, EPS)
        zero_t = const.tile([128, 1], F32)
        nc.vector.memset(zero_t, 0.0)
        magic_t = const.tile([128, 1], mybir.dt.int32)
        nc.vector.memset(magic_t, 0x5f3759df)
        one_i = const.tile([128, 1], mybir.dt.int32)
        nc.vector.memset(one_i, 1)
        neghalf_t = const.tile([128, 1], F32)
        nc.vector.memset(neghalf_t, -0.5)
        threehalf_t = const.tile([128, 1], F32)
        nc.vector.memset(threehalf_t, 1.5)
        negone_t = const.tile([128, 1], F32)
        nc.vector.memset(negone_t, -1.0)
        I32 = mybir.dt.int32

        def emit_rstd(pool, var_ap, nm):
            """1/sqrt(var+EPS) on DVE only (bit trick + 2 Newton iters)."""
            scr = pool.tile([128, 5], F32, tag="scr", name=f"scr{nm}")
            vpe, y = scr[:, 0:1], scr[:, 1:2]
            y2, b = scr[:, 2:3], scr[:, 3:4]
            d = scr[:, 4:5]
            nc.vector.tensor_tensor(vpe, var_ap, eps_t, op=AluOpType.add)
            nc.vector.tensor_scalar(
                out=y.bitcast(I32), in0=vpe.bitcast(I32), scalar1=one_i,
                scalar2=None, op0=AluOpType.logical_shift_right)
            nc.vector.tensor_tensor(out=y.bitcast(I32), in0=magic_t,
                                    in1=y.bitcast(I32), op=AluOpType.subtract)
            for _ in range(2):
                nc.vector.tensor_tensor(y2, y, y, op=AluOpType.mult)
                nc.vector.tensor_tensor(b, vpe, y2, op=AluOpType.mult)
                nc.vector.scalar_tensor_tensor(
                    out=d, in0=b, scalar=neghalf_t, in1=threehalf_t,
                    op0=AluOpType.mult, op1=AluOpType.add)
                nc.vector.tensor_tensor(y, y, d, op=AluOpType.mult)
            return y

        ones_t = const.tile([1, S], BF16)
        nc.gpsimd.dma_start(ones_t, ones_d.ap())
        iden_t = const.tile([128, 128], BF16)
        nc.gpsimd.dma_start(iden_t, iden_d.ap())
        if have_gb:
            g1_b = const.tile([128, D], F32)
            nc.sync.dma_start(g1_b, g1_d.ap().partition_broadcast(128))
            be1_b = const.tile([128, D], F32)
            nc.sync.dma_start(be1_b, be1_d.ap().partition_broadcast(128))
        if have_bkq:
            bkq_t = const.tile([1, 128], BF16)
            nc.sync.dma_start(bkq_t, bkq_d.ap())
        if have_bb:
            bb_t = const.tile([1, 1], BF16)
            nc.sync.dma_start(bb_t, bb_d.ap())
        if have_bv:
            bvs_t = const.tile([1, D], BF16)
            nc.sync.dma_start(bvs_t, bvs_d.ap())
            bvcs_t = const.tile([1, D], BF16)
            nc.sync.dma_start(bvcs_t, bvcs_d.ap())
        if have_b1:
            b1_t = const.tile([1, D], BF16)
            nc.sync.dma_start(b1_t, b1_d.ap())

        # ---- kq/bias output tiles (rows 0..64)
        tile_k = kqp.tile([65, S], BF16)   # rows0-63 keysT*(s/2), row64 biasT/2
        tile_q = kqp.tile([65, S], BF16)   # rows0-63 queriesT, row64 ones
        nc.gpsimd.dma_start(tile_q[64:65, :], ones_d.ap())

        # ---- vh: values/2 s-major fp8, resident through phase 1
        vh_sb = vp.tile([128, NC, D], F8E4)
        # ---- res: LN1 output, bf16, resident through phase 2
        res_b = resp.tile([128, NC, D], BF16)
        # ---- per-row score centers and colsum correction row
        mB = csp.tile([128, S], BF16)        # broadcast of m = tanh(bias/2)
        onepm = csp.tile([1, S], BF16)       # 1 + m
        csrow = csp.tile([1, D], BF16)       # colsum(vh) exact
        one_f = const.tile([1, 1], F32)
        nc.vector.memset(one_f, 1.0)

        # ================= phase 0: projections =================
        with ExitStack() as ph0:
            xp = ph0.enter_context(tc.tile_pool(name="xp", bufs=1))
            wp = ph0.enter_context(tc.tile_pool(name="wp", bufs=1))

            # weights first (small, needed immediately), then x^T in
            # half-chunks ordered to match kq consumption (h=0 first)
            wkq_t = xp.tile([128, 8, 128], BF16)
            nc.sync.dma_start(wkq_t, wkq_d.ap())
            wb_t = xp.tile([128, 8, 1], BF16)
            nc.scalar.dma_start(wb_t, wb_d.ap())
            xt_t = xp.tile([128, 8, S], BF16)
            qs = [nc.sync, nc.scalar, nc.gpsimd]
            for h in range(2):
                for k in range(8):
                    sl = slice(h * 1024, (h + 1) * 1024)
                    qs[k % 3].dma_start(xt_t[:, k, sl], xb_d[:, k, sl])
            wv_t = wp.tile([128, 8, D], BF16, tag="wv")
            nc.gpsimd.dma_start(wv_t[:, 0:2, :], wv_d[:, 0:2, :])
            nc.gpsimd.dma_start(wv_t[:, 2:4, :], wv_d[:, 2:4, :])
            nc.gpsimd.dma_start(wv_t[:, 4:6, :], wv_d[:, 4:6, :])
            nc.gpsimd.dma_start(wv_t[:, 6:8, :], wv_d[:, 6:8, :])

            # xsum[d] = sum_s xT[d, s]  (for the exact colsum correction),
            # split per chunk so each runs as soon as its DMA lands
            xsum_f = xp.tile([128, 8], F32, tag="xsf")
            for k in range(8):
                nc.vector.tensor_reduce(xsum_f[:, k:k + 1], xt_t[:, k, :],
                                        axis=mybir.AxisListType.X,
                                        op=AluOpType.add)
            xsum_b = xp.tile([128, 8], BF16, tag="xsb")
            nc.vector.tensor_copy(xsum_b, xsum_f)

            # kq + bias, k-outer in two passes (PSUM: pkq 2 + pb 2 banks)
            ph0kq = ph0.enter_context(ExitStack())
            pkq = ph0kq.enter_context(
                tc.tile_pool(name="pkq", bufs=2, space="PSUM"))
            pb = ph0kq.enter_context(
                tc.tile_pool(name="pb", bufs=2, space="PSUM"))
            for half in range(2):
                sts = (0, 1) if half == 0 else (2, 3)
                pks = {st: pkq.tile([128, 512], F32, tag="kq",
                                    name=f"pk{st}") for st in sts}
                pbs = {st: pb.tile([1, 512], F32, tag="b",
                                   name=f"pb{st}") for st in sts}
                for k in range(8):
                    for st in sts:   # consecutive pair shares stationary
                        sl = slice(st * SB, (st + 1) * SB)
                        nc.tensor.matmul(pks[st], wkq_t[:, k, :],
                                         xt_t[:, k, sl], start=(k == 0),
                                         stop=(k == 7 and not have_bkq))
                for k in range(8):
                    for st in sts:
                        sl = slice(st * SB, (st + 1) * SB)
                        nc.tensor.matmul(pbs[st], wb_t[:, k, :],
                                         xt_t[:, k, sl], start=(k == 0),
                                         stop=(k == 7 and not have_bb))
                for st in sts:
                    sl = slice(st * SB, (st + 1) * SB)
                    if have_bkq:
                        nc.tensor.matmul(pks[st], bkq_t, ones_t[:, sl],
                                         start=False, stop=True)
                    if have_bb:
                        nc.tensor.matmul(pbs[st], bb_t, ones_t[:, sl],
                                         start=False, stop=True)
                    nc.scalar.activation(tile_k[0:64, sl], pks[st][0:64, :],
                                         ACTF.Copy)
                    nc.vector.tensor_copy(tile_q[0:64, sl],
                                          pks[st][64:128, :])
                    nc.scalar.activation(tile_k[64:65, sl], pbs[st], ACTF.Copy)

            # per-row centers m = tanh(biasT/2); broadcast via DRAM roundtrip
            m_row = xp.tile([1, S], BF16, tag="mrow")
            nc.scalar.activation(m_row, tile_k[64:65, :], ACTF.Tanh)
            nc.vector.tensor_scalar(out=onepm, in0=m_row, scalar1=one_f,
                                    scalar2=None, op0=AluOpType.add)
            nc.sync.dma_start(m_spill.ap(), m_row)
            nc.sync.dma_start(mB, m_spill.ap().partition_broadcast(128))

            # colsum(vh) = xsum @ (Wv/2) (+ S*bv/2), kept as bf16 row
            ph0kq.close()
            pcs = ph0.enter_context(
                tc.tile_pool(name="pcs", bufs=2, space="PSUM"))
            pcst = {dt2: pcs.tile([1, 512], F32, tag="cs", name=f"cs{dt2}")
                    for dt2 in range(ND2)}
            for k in range(8):
                for dt2 in range(ND2):
                    dsl = slice(dt2 * 512, (dt2 + 1) * 512)
                    nc.tensor.matmul(pcst[dt2], xsum_b[:, k:k + 1],
                                     wv_t[:, k, dsl], start=(k == 0),
                                     stop=(k == 7 and not have_bv))
            if have_bv:
                for dt2 in range(ND2):
                    dsl = slice(dt2 * 512, (dt2 + 1) * 512)
                    nc.tensor.matmul(pcst[dt2], ones_t[0:1, 0:1],
                                     bvcs_t[:, dsl], start=False, stop=True)
            for dt2 in range(ND2):
                nc.scalar.activation(csrow[:, dt2 * 512:(dt2 + 1) * 512],
                                     pcst[dt2], ACTF.Copy)

            # values/2: out[s-chunk,128 x d-512] -> fp8, stationary xT reused
            pv = ph0.enter_context(
                tc.tile_pool(name="pv", bufs=4, space="PSUM"))
            for sc in range(NC):
                pvt = {dt2: pv.tile([128, 512], F32, tag="v", name=f"pv{dt2}")
                       for dt2 in range(ND2)}
                for k in range(8):
                    for dt2 in range(ND2):   # pair shares stationary
                        nc.tensor.matmul(
                            pvt[dt2], xt_t[:, k, sc * 128:(sc + 1) * 128],
                            wv_t[:, k, dt2 * 512:(dt2 + 1) * 512],
                            start=(k == 0), stop=(k == 7 and not have_bv))
                if have_bv:
                    for dt2 in range(ND2):
                        nc.tensor.matmul(
                            pvt[dt2], ones_t[:, 0:128],
                            bvs_t[:, dt2 * 512:(dt2 + 1) * 512],
                            start=False, stop=True)
                for dt2 in range(ND2):
                    nc.scalar.activation(
                        vh_sb[:, sc, dt2 * 512:(dt2 + 1) * 512], pvt[dt2],
                        ACTF.Copy)

        # ================= phase 1: scores + attention + LN1 =================
        w1p = top.enter_context(tc.tile_pool(name="w1p", bufs=1))
        w1_t = w1p.tile([128, 8, D], BF16)
        nc.scalar.dma_start(w1_t[:, 0:4, :], w1_d[:, 0:4, :])
        nc.gpsimd.dma_start(w1_t[:, 4:8, :], w1_d[:, 4:8, :])

        with ExitStack() as ph1i:
            strips = ph1i.enter_context(tc.tile_pool(name="strips", bufs=8))
            tpool = ph1i.enter_context(tc.tile_pool(name="tpool", bufs=3))
            xin = ph1i.enter_context(tc.tile_pool(name="xin", bufs=3))
            rpre = ph1i.enter_context(tc.tile_pool(name="rpre", bufs=3))
            stat = ph1i.enter_context(tc.tile_pool(name="stat", bufs=6))
            ps_s = ph1i.enter_context(
                tc.tile_pool(name="ps_s", bufs=4, space="PSUM"))
            ps_a = ph1i.enter_context(
                tc.tile_pool(name="ps_a", bufs=4, space="PSUM"))

            all_halves = {}

            def emit_scores(sb, jlist):
                """z/2 matmul + tanh + per-row centering -> fp8 strips."""
                isl = slice(sb * SB, (sb + 1) * SB)
                if sb not in all_halves:
                    all_halves[sb] = [
                        strips.tile([128, 4, SB], F8E4, tag="strip",
                                    name=f"strip_{sb}_{q}")
                        for q in range(4)]
                halves = all_halves[sb]
                for j in jlist:
                    pst = ps_s.tile([128, SB], F32, tag="s")
                    nc.tensor.matmul(
                        pst, tile_q[:, j * 128:(j + 1) * 128],
                        tile_k[:, isl], start=True, stop=True)
                    tt = tpool.tile([128, SB], F32, tag="tt",
                                    name=f"tt{sb}_{j % 3}")
                    nc.scalar.activation(tt, pst, ACTF.Tanh)
                    nc.vector.tensor_tensor(
                        halves[j // 4][:, j % 4, :], tt, mB[:, isl],
                        op=AluOpType.subtract)

            emit_scores(0, range(NC))
            for sb in range(NSB):
                halves = all_halves.pop(sb)
                for l in range(4):
                    # produce next superblock's strips while attention runs
                    if sb + 1 < NSB:
                        emit_scores(sb + 1, range(l * 4, (l + 1) * 4))
                    c = sb * 4 + l   # global 128-row chunk
                    x_t = xin.tile([128, D], F32, tag="x", name=f"x{l}")
                    nc.gpsimd.dma_start(x_t, x_d[c * 128:(c + 1) * 128, :])
                    pa = [ps_a.tile([128, 512], F32, tag="a", name=f"pa{_d}")
                          for _d in range(ND2)]
                    csl = slice(c * 128, (c + 1) * 128)
                    for dt2 in range(ND2):
                        # rank-1 seed: (1+m_i) * colsum(vh)[d]
                        nc.tensor.matmul(
                            pa[dt2], onepm[0:1, csl],
                            csrow[0:1, dt2 * 512:(dt2 + 1) * 512],
                            start=True, stop=False)
                    for jp in range(8):      # DoubleRow over j-chunk pairs
                        q, m = jp // 2, (jp % 2) * 2
                        st_ap = halves[q][:, m:m + 2, l * 128:(l + 1) * 128]
                        for dt2 in range(ND2):   # pair shares stationary
                            nc.tensor.matmul(
                                pa[dt2], st_ap,
                                vh_sb[:, 2 * jp:2 * jp + 2,
                                      dt2 * 512:(dt2 + 1) * 512],
                                start=False, stop=(jp == 7),
                                perf_mode=DRMODE)
                    rp = rpre.tile([128, D], F32, tag="rp", name=f"rp{l}")
                    for dt2 in range(ND2):
                        dsl = slice(dt2 * 512, (dt2 + 1) * 512)
                        nc.vector.tensor_tensor(
                            rp[:, dsl], pa[dt2], x_t[:, dsl],
                            op=AluOpType.add)
                    st_t = stat.tile([128, 2, 6], F32, tag="bst",
                                     name=f"bst{l}")
                    for g in range(2):
                        nc.vector.bn_stats(st_t[:, g, :],
                                           rp[:, g * 512:(g + 1) * 512])
                    mv = stat.tile([128, 2], F32, tag="mv", name=f"mv{l}")
                    nc.vector.bn_aggr(mv, st_t)
                    rstd = emit_rstd(stat, mv[:, 1:2], f"r1_{c}")
                    if have_gb:
                        t1 = rpre.tile([128, D], F32, tag="t1", name=f"t1{l}")
                        nc.vector.scalar_tensor_tensor(
                            out=t1, in0=rp, scalar=mv[:, 0:1],
                            in1=g1_b,
                            op0=AluOpType.subtract, op1=AluOpType.mult)
                        nc.vector.scalar_tensor_tensor(
                            out=res_b[:, c, :], in0=t1, scalar=rstd,
                            in1=be1_b,
                            op0=AluOpType.mult, op1=AluOpType.add)
                    else:
                        nc.vector.tensor_scalar(
                            out=res_b[:, c, :], in0=rp, scalar1=mv[:, 0:1],
                            scalar2=rstd,
                            op0=AluOpType.subtract,
                            op1=AluOpType.mult)

        # ================= phase 2: FF + LN2 =================
        with ExitStack() as ph2:
            rtp = ph2.enter_context(tc.tile_pool(name="rtp", bufs=4))
            f2 = ph2.enter_context(tc.tile_pool(name="f2", bufs=2))
            ostage = ph2.enter_context(tc.tile_pool(name="ostage", bufs=2))
            stat2 = ph2.enter_context(tc.tile_pool(name="stat2", bufs=6))
            ps_t = ph2.enter_context(
                tc.tile_pool(name="ps_t", bufs=4, space="PSUM"))
            ps_f = ph2.enter_context(
                tc.tile_pool(name="ps_f", bufs=4, space="PSUM"))

            rT = [None] * NC

            def stage_tr(c):
                rt_ = rtp.tile([128, 8, 128], BF16, tag="rT")
                for h in range(2):
                    ptr = ps_t.tile([128, 512], BF16, tag="tr",
                                    name=f"ptr{h}")
                    for k in range(4):
                        kk = h * 4 + k
                        nc.tensor.transpose(
                            ptr[:, k * 128:(k + 1) * 128],
                            res_b[:, c, kk * 128:(kk + 1) * 128], iden_t)
                    nc.scalar.activation(rt_[:, h * 4:(h + 1) * 4, :], ptr,
                                         ACTF.Copy)
                rT[c] = rt_

            def stage_ff(c):
                pf = {dt2: ps_f.tile([128, 512], F32, tag="f", name=f"pf{_d}")
                      for dt2, _d in ((d_, d_) for d_ in range(ND2))}
                for k in range(8):
                    for dt2 in range(ND2):   # pair shares stationary
                        dsl = slice(dt2 * 512, (dt2 + 1) * 512)
                        nc.tensor.matmul(pf[dt2], rT[c][:, k, :],
                                         w1_t[:, k, dsl],
                                         start=(k == 0),
                                         stop=(k == 7 and not have_b1))
                if have_b1:
                    for dt2 in range(ND2):
                        dsl = slice(dt2 * 512, (dt2 + 1) * 512)
                        nc.tensor.matmul(pf[dt2], ones_t[0:1, 0:128],
                                         b1_t[:, dsl], start=False, stop=True)
                r2 = f2.tile([128, D], F32, tag="r2")
                for dt2 in range(ND2):
                    dsl = slice(dt2 * 512, (dt2 + 1) * 512)
                    nc.vector.scalar_tensor_tensor(
                        out=r2[:, dsl], in0=pf[dt2], scalar=zero_t,
                        in1=res_b[:, c, dsl],
                        op0=AluOpType.max, op1=AluOpType.add)
                st_t = stat2.tile([128, 2, 6], F32, tag="bst2")
                for g in range(2):
                    nc.vector.bn_stats(st_t[:, g, :],
                                       r2[:, g * 512:(g + 1) * 512])
                mv = stat2.tile([128, 2], F32, tag="mv2")
                nc.vector.bn_aggr(mv, st_t)
                rstd = emit_rstd(stat2, mv[:, 1:2], f"r2_{c}")
                o_t = ostage.tile([128, D], F32, tag="o")
                nc.vector.tensor_scalar(
                    out=o_t, in0=r2, scalar1=mv[:, 0:1], scalar2=rstd,
                    op0=AluOpType.subtract, op1=AluOpType.mult)
                nc.sync.dma_start(out_d[c * 128:(c + 1) * 128, :], o_t)

            for c in range(NC + 2):
                if c < NC:
                    stage_tr(c)
                if c >= 2:
                    stage_ff(c - 2)

    nc.finalize()
    return nc


_PROGRAM_CACHE = {}


def _get_program(flags):
    if flags not in _PROGRAM_CACHE:
        _PROGRAM_CACHE[flags] = build_program(flags)
    return _PROGRAM_CACHE[flags]


def kernel(x, Wk, bk, Wq, bq, Wv, bv, Wb, bb, W1, b1, g1, be1):
    import math
    scale = 1.0 / math.sqrt(K)
    flags = (
        bool(np.any(bk) or np.any(bq)),
        bool(np.any(bb)),
        bool(np.any(bv)),
        bool(np.any(b1)),
        bool(np.any(g1 != 1.0) or np.any(be1)),
    )
    nc = _get_program(flags)

    def _prep(w):
        # [D, M] -> [128, 8, M] matching SBUF [partition, chunk, free], bf16
        return np.ascontiguousarray(
            w.astype(np.float32).reshape(8, 128, -1).transpose(1, 0, 2)
        ).astype(ml_dtypes.bfloat16)

    # z/2 trick: keys side gets scale/2, bias side gets 1/2
    wkq = _prep(np.concatenate([Wk * (scale * 0.5), Wq], axis=1))
    wb = _prep(np.asarray(Wb) * 0.5)
    wv = _prep(np.asarray(Wv) * 0.5)
    w1 = _prep(np.asarray(W1))
    onesrow = np.ones((1, S), dtype=ml_dtypes.bfloat16)
    iden = np.eye(128, dtype=ml_dtypes.bfloat16)
    bkq = np.concatenate([np.asarray(bk) * (scale * 0.5), np.asarray(bq)]
                         )[None, :].astype(ml_dtypes.bfloat16)
    bbr = (np.asarray(bb, dtype=np.float32) * 0.5).reshape(1, 1).astype(
        ml_dtypes.bfloat16)
    bvsr = (np.asarray(bv, dtype=np.float32) * 0.5)[None, :].astype(
        ml_dtypes.bfloat16)
    bvcsr = (np.asarray(bv, dtype=np.float32) * (0.5 * S))[None, :].astype(
        ml_dtypes.bfloat16)
    b1r = np.asarray(b1, dtype=np.float32)[None, :].astype(ml_dtypes.bfloat16)
    g1r = np.asarray(g1, dtype=np.float32)[None, :]
    be1r = np.asarray(be1, dtype=np.float32)[None, :]

    xb = np.ascontiguousarray(np.transpose(x, (0, 2, 1)))        # [B,D,S]
    xb = np.ascontiguousarray(
        xb.reshape(B, 8, 128, S).transpose(0, 2, 1, 3)            # [B,128,8,S]
    ).astype(ml_dtypes.bfloat16)

    in_maps = []
    for b in range(B):
        in_maps.append(dict(
            xb=xb[b], x=np.ascontiguousarray(x[b], dtype=np.float32),
            wkq=wkq, wb=wb, wv=wv, w1=w1, onesrow=onesrow, iden=iden,
            g1=g1r, be1=be1r, bkq=bkq, bb=bbr, bvs=bvsr, bvcs=bvcsr,
            b1=b1r))

    res = run_bass_kernel_spmd(nc, in_maps, list(range(NCORES)), trace=False)
    out = np.stack([res.results[b]["out"] for b in range(B)], axis=0)
    return out.astype(np.float32)


# revision 31
# speedup vs baseline: 1.1499x; 1.0555x over previous
"""AttentionEncoder TRN2 Bass kernel (v2: fp8 DoubleRow attention + bf16 pipeline).

Data-parallel over batch: B=8 samples -> 8 NeuronCores, one sample per core.

Math identity used for the attention matmul (the dominant 4.3 GMAC):
  scores = sigmoid(z), z = qk + bias
  attention = scores @ v = tanh(z/2) @ (v/2) + colsum(v/2)
The tanh term runs as fp8e4 DoubleRow matmuls (2x MACs/instr); centering via
tanh makes the fp8 quantization of scores ~3x finer, and computing the
colsum correction exactly (colsum(v/2) = xsum @ (Wv/2) + S*bv/2, xsum from a
DVE free-dim reduce of x^T) cancels the mean-term of the fp8 value
quantization error. z/2 is produced directly by folding 0.5 into Wk/Wb on
host. All other matmuls run in bf16 (same PE rate as fp32r but weight loads
fully hidden); res stays resident in SBUF as bf16 (no DRAM spill).

Per-core phases (S=2048, D=1024, K=64):
  phase 0: kq^T = [Wk*s/2|Wq]^T@x^T (+bias row trick), colsum via xsum@Wvh,
           vh = x@(Wv/2) s-major -> fp8e4
  phase 1: per 512-col superblock: z/2 matmuls -> tanh -> fp8 strips;
           attention += strips(DR pairs)@vh; epilogue rp=pa+(x+colsumB),
           LN1 via bn_stats -> res_bf16 resident
  phase 2: per 128-row chunk: PE-transpose res, FF matmul bf16,
           relu+residual, LN2, DMA out.
"""
import numpy as np
import ml_dtypes
from contextlib import ExitStack

import concourse.bass as bass
import concourse.tile as tile
from concourse import bacc, mybir
from concourse.bass_utils import run_bass_kernel_spmd
from concourse.alu_op_type import AluOpType

F32 = mybir.dt.float32
BF16 = mybir.dt.bfloat16
F8E4 = mybir.dt.float8e4
ACTF = mybir.ActivationFunctionType
DRMODE = mybir.MatmulPerfMode.DoubleRow

B, S, D, K = 8, 2048, 1024, 64
EPS = 1e-5
NCORES = 8
SB = 512          # superblock width (scores free dim)
NSB = S // SB     # 4
NC = S // 128     # 16 s-chunks
ND2 = D // 512    # 2 d-tiles


def build_program(flags):
    have_bkq, have_bb, have_bv, have_b1, have_gb = flags
    nc = bacc.Bacc(trn_type="TRN2")

    xb_d = nc.declare_dram_parameter("xb", [128, 8, S], BF16, isOutput=False)
    x_d = nc.declare_dram_parameter("x", [S, D], F32, isOutput=False)
    wkq_d = nc.declare_dram_parameter("wkq", [128, 8, 128], BF16, isOutput=False)
    wb_d = nc.declare_dram_parameter("wb", [128, 8, 1], BF16, isOutput=False)
    wv_d = nc.declare_dram_parameter("wv", [128, 8, D], BF16, isOutput=False)
    w1_d = nc.declare_dram_parameter("w1", [128, 8, D], BF16, isOutput=False)
    ones_d = nc.declare_dram_parameter("onesrow", [1, S], BF16, isOutput=False)
    iden_d = nc.declare_dram_parameter("iden", [128, 128], BF16, isOutput=False)
    g1_d = nc.declare_dram_parameter("g1", [1, D], F32, isOutput=False)
    be1_d = nc.declare_dram_parameter("be1", [1, D], F32, isOutput=False)
    bkq_d = nc.declare_dram_parameter("bkq", [1, 128], BF16, isOutput=False)
    bb_d = nc.declare_dram_parameter("bb", [1, 1], BF16, isOutput=False)
    bvs_d = nc.declare_dram_parameter("bvs", [1, D], BF16, isOutput=False)
    bvcs_d = nc.declare_dram_parameter("bvcs", [1, D], BF16, isOutput=False)
    b1_d = nc.declare_dram_parameter("b1", [1, D], BF16, isOutput=False)
    out_d = nc.declare_dram_parameter("out", [S, D], F32, isOutput=True)

    m_spill = nc.dram_tensor("m_spill", [1, S], BF16)

    with tile.TileContext(nc) as tc, ExitStack() as top:
        const = top.enter_context(tc.tile_pool(name="const", bufs=1))
        kqp = top.enter_context(tc.tile_pool(name="kqp", bufs=1))
        vp = top.enter_context(tc.tile_pool(name="vp", bufs=1))
        resp = top.enter_context(tc.tile_pool(name="resp", bufs=1))
        csp = top.enter_context(tc.tile_pool(name="csp", bufs=1))

        # ---- constants
        eps_t = const.tile([128, 1], F32)
        nc.vector.memset(eps_t, EPS)
        zero_t = const.tile([128, 1], F32)
        nc.vector.memset(zero_t, 0.0)
        magic_t = const.tile([128, 1], mybir.dt.int32)
        nc.vector.memset(magic_t, 0x5f3759df)
        one_i = const.tile([128, 1], mybir.dt.int32)
        nc.vector.memset(one_i, 1)
        neghalf_t = const.tile([128, 1], F32)
        nc.vector.memset(neghalf_t, -0.5)
        threehalf_t = const.tile([128, 1], F32)
        nc.vector.memset(threehalf_t, 1.5)
        negone_t = const.tile([128, 1], F32)
        nc.vector.memset(negone_t, -1.0)
        I32 = mybir.dt.int32

        def emit_rstd(pool, var_ap, nm):
            """1/sqrt(var+EPS) on DVE only (bit trick + 2 Newton iters)."""
            scr = pool.tile([128, 5], F32, tag="scr", name=f"scr{nm}")
            vpe, y = scr[:, 0:1], scr[:, 1:2]
            y2, b = scr[:, 2:3], scr[:, 3:4]
            d = scr[:, 4:5]
            nc.vector.tensor_tensor(vpe, var_ap, eps_t, op=AluOpType.add)
            nc.vector.tensor_scalar(
                out=y.bitcast(I32), in0=vpe.bitcast(I32), scalar1=one_i,
                scalar2=None, op0=AluOpType.logical_shift_right)
            nc.vector.tensor_tensor(out=y.bitcast(I32), in0=magic_t,
                                    in1=y.bitcast(I32), op=AluOpType.subtract)
            for _ in range(2):
                nc.vector.tensor_tensor(y2, y, y, op=AluOpType.mult)
                nc.vector.tensor_tensor(b, vpe, y2, op=AluOpType.mult)
                nc.vector.scalar_tensor_tensor(
                    out=d, in0=b, scalar=neghalf_t, in1=threehalf_t,
                    op0=AluOpType.mult, op1=AluOpType.add)
                nc.vector.tensor_tensor(y, y, d, op=AluOpType.mult)
            return y

        def emit_rstd_fast(pool, var_ap, nm):
            """1/sqrt(var+EPS): DVE add+reciprocal, scalar sqrt (3 ops)."""
            scr = pool.tile([128, 2], F32, tag="scr", name=f"scr{nm}")
            vpe, inv = scr[:, 0:1], scr[:, 1:2]
            nc.vector.tensor_tensor(vpe, var_ap, eps_t, op=AluOpType.add)
            nc.vector.reciprocal(inv, vpe)
            rstd = pool.tile([128, 1], F32, tag="rstd", name=f"rstd{nm}")
            nc.scalar.activation(rstd, inv, ACTF.Sqrt)
            return rstd

        ones_t = const.tile([1, S], BF16)
        nc.gpsimd.dma_start(ones_t, ones_d.ap())
        iden_t = const.tile([128, 128], BF16)
        nc.gpsimd.dma_start(iden_t, iden_d.ap())
        if have_gb:
            g1_b = const.tile([128, D], F32)
            nc.sync.dma_start(g1_b, g1_d.ap().partition_broadcast(128))
            be1_b = const.tile([128, D], F32)
            nc.sync.dma_start(be1_b, be1_d.ap().partition_broadcast(128))
        if have_bkq:
            bkq_t = const.tile([1, 128], BF16)
            nc.sync.dma_start(bkq_t, bkq_d.ap())
        if have_bb:
            bb_t = const.tile([1, 1], BF16)
            nc.sync.dma_start(bb_t, bb_d.ap())
        if have_bv:
            bvs_t = const.tile([1, D], BF16)
            nc.sync.dma_start(bvs_t, bvs_d.ap())
            bvcs_t = const.tile([1, D], BF16)
            nc.sync.dma_start(bvcs_t, bvcs_d.ap())
        if have_b1:
            b1_t = const.tile([1, D], BF16)
            nc.sync.dma_start(b1_t, b1_d.ap())

        # ---- kq/bias output tiles (rows 0..64)
        tile_k = kqp.tile([65, S], BF16)   # rows0-63 keysT*(s/2), row64 biasT/2
        tile_q = kqp.tile([65, S], BF16)   # rows0-63 queriesT, row64 ones
        nc.gpsimd.dma_start(tile_q[64:65, :], ones_d.ap())

        # ---- vh: values/2 s-major fp8, resident through phase 1
        vh_sb = vp.tile([128, NC, D], F8E4)
        # ---- res: LN1 output, bf16, resident through phase 2
        res_b = resp.tile([128, NC, D], BF16)
        # ---- per-row score centers and colsum correction row
        mB = csp.tile([128, S], BF16)        # broadcast of m = tanh(bias/2)
        onepm = csp.tile([1, S], BF16)       # 1 + m
        csrow = csp.tile([1, D], BF16)       # colsum(vh) exact
        one_f = const.tile([1, 1], F32)
        nc.vector.memset(one_f, 1.0)

        # ================= phase 0: projections =================
        with ExitStack() as ph0:
            xp = ph0.enter_context(tc.tile_pool(name="xp", bufs=1))
            wp = ph0.enter_context(tc.tile_pool(name="wp", bufs=1))

            # weights first (small, needed immediately), then x^T in
            # half-chunks ordered to match kq consumption (h=0 first)
            wkq_t = xp.tile([128, 8, 128], BF16)
            nc.sync.dma_start(wkq_t, wkq_d.ap())
            wb_t = xp.tile([128, 8, 1], BF16)
            nc.scalar.dma_start(wb_t, wb_d.ap())
            xt_t = xp.tile([128, 8, S], BF16)
            qs = [nc.sync, nc.scalar, nc.gpsimd]
            for h in range(2):
                for k in range(8):
                    sl = slice(h * 1024, (h + 1) * 1024)
                    qs[k % 3].dma_start(xt_t[:, k, sl], xb_d[:, k, sl])
            wv_t = wp.tile([128, 8, D], BF16, tag="wv")
            nc.gpsimd.dma_start(wv_t[:, 0:2, :], wv_d[:, 0:2, :])
            nc.gpsimd.dma_start(wv_t[:, 2:4, :], wv_d[:, 2:4, :])
            nc.gpsimd.dma_start(wv_t[:, 4:6, :], wv_d[:, 4:6, :])
            nc.gpsimd.dma_start(wv_t[:, 6:8, :], wv_d[:, 6:8, :])

            # xsum[d] = sum_s xT[d, s]  (for the exact colsum correction),
            # split per chunk so each runs as soon as its DMA lands
            xsum_f = xp.tile([128, 8], F32, tag="xsf")
            for k in range(8):
                nc.vector.tensor_reduce(xsum_f[:, k:k + 1], xt_t[:, k, :],
                                        axis=mybir.AxisListType.X,
                                        op=AluOpType.add)
            xsum_b = xp.tile([128, 8], BF16, tag="xsb")
            nc.vector.tensor_copy(xsum_b, xsum_f)

            # kq + bias, k-outer in two passes (PSUM: pkq 2 + pb 2 banks)
            ph0kq = ph0.enter_context(ExitStack())
            pkq = ph0kq.enter_context(
                tc.tile_pool(name="pkq", bufs=2, space="PSUM"))
            pb = ph0kq.enter_context(
                tc.tile_pool(name="pb", bufs=2, space="PSUM"))
            for half in range(2):
                sts = (0, 1) if half == 0 else (2, 3)
                pks = {st: pkq.tile([128, 512], F32, tag="kq",
                                    name=f"pk{st}") for st in sts}
                pbs = {st: pb.tile([1, 512], F32, tag="b",
                                   name=f"pb{st}") for st in sts}
                for k in range(8):
                    for st in sts:   # consecutive pair shares stationary
                        sl = slice(st * SB, (st + 1) * SB)
                        nc.tensor.matmul(pks[st], wkq_t[:, k, :],
                                         xt_t[:, k, sl], start=(k == 0),
                                         stop=(k == 7 and not have_bkq))
                for k in range(8):
                    for st in sts:
                        sl = slice(st * SB, (st + 1) * SB)
                        nc.tensor.matmul(pbs[st], wb_t[:, k, :],
                                         xt_t[:, k, sl], start=(k == 0),
                                         stop=(k == 7 and not have_bb))
                for st in sts:
                    sl = slice(st * SB, (st + 1) * SB)
                    if have_bkq:
                        nc.tensor.matmul(pks[st], bkq_t, ones_t[:, sl],
                                         start=False, stop=True)
                    if have_bb:
                        nc.tensor.matmul(pbs[st], bb_t, ones_t[:, sl],
                                         start=False, stop=True)
                    nc.scalar.activation(tile_k[0:64, sl], pks[st][0:64, :],
                                         ACTF.Copy)
                    nc.vector.tensor_copy(tile_q[0:64, sl],
                                          pks[st][64:128, :])
                    nc.scalar.activation(tile_k[64:65, sl], pbs[st], ACTF.Copy)

            # per-row centers m = tanh(biasT/2); broadcast via DRAM roundtrip
            m_row = xp.tile([1, S], BF16, tag="mrow")
            nc.scalar.activation(m_row, tile_k[64:65, :], ACTF.Tanh)
            nc.vector.tensor_scalar(out=onepm, in0=m_row, scalar1=one_f,
                                    scalar2=None, op0=AluOpType.add)
            nc.sync.dma_start(m_spill.ap(), m_row)
            nc.sync.dma_start(mB, m_spill.ap().partition_broadcast(128))

            # values/2: out[s-chunk,128 x d-512] -> fp8, stationary xT reused
            ph0kq.close()
            pv = ph0.enter_context(
                tc.tile_pool(name="pv", bufs=4, space="PSUM"))
            for sc in range(NC):
                pvt = {dt2: pv.tile([128, 512], F32, tag="v", name=f"pv{dt2}")
                       for dt2 in range(ND2)}
                for k in range(8):
                    for dt2 in range(ND2):   # pair shares stationary
                        nc.tensor.matmul(
                            pvt[dt2], xt_t[:, k, sc * 128:(sc + 1) * 128],
                            wv_t[:, k, dt2 * 512:(dt2 + 1) * 512],
                            start=(k == 0), stop=(k == 7 and not have_bv))
                if have_bv:
                    for dt2 in range(ND2):
                        nc.tensor.matmul(
                            pvt[dt2], ones_t[:, 0:128],
                            bvs_t[:, dt2 * 512:(dt2 + 1) * 512],
                            start=False, stop=True)
                for dt2 in range(ND2):
                    nc.scalar.activation(
                        vh_sb[:, sc, dt2 * 512:(dt2 + 1) * 512], pvt[dt2],
                        ACTF.Copy)

            # colsum(vh) = xsum @ (Wv/2) (+ S*bv/2), kept as bf16 row
            # (placed after values so the xsum reduce never stalls the PE)
            pcs = ph0.enter_context(
                tc.tile_pool(name="pcs", bufs=2, space="PSUM"))
            pcst = {dt2: pcs.tile([1, 512], F32, tag="cs", name=f"cs{dt2}")
                    for dt2 in range(ND2)}
            for k in range(8):
                for dt2 in range(ND2):
                    dsl = slice(dt2 * 512, (dt2 + 1) * 512)
                    nc.tensor.matmul(pcst[dt2], xsum_b[:, k:k + 1],
                                     wv_t[:, k, dsl], start=(k == 0),
                                     stop=(k == 7 and not have_bv))
            if have_bv:
                for dt2 in range(ND2):
                    dsl = slice(dt2 * 512, (dt2 + 1) * 512)
                    nc.tensor.matmul(pcst[dt2], ones_t[0:1, 0:1],
                                     bvcs_t[:, dsl], start=False, stop=True)
            for dt2 in range(ND2):
                nc.scalar.activation(csrow[:, dt2 * 512:(dt2 + 1) * 512],
                                     pcst[dt2], ACTF.Copy)

        # ============ phase 1+2 merged: scores/attention/LN1/FF/LN2 ==========
        w1p = top.enter_context(tc.tile_pool(name="w1p", bufs=1))
        w1_t = w1p.tile([128, 8, D], BF16)
        nc.scalar.dma_start(w1_t[:, 0:4, :], w1_d[:, 0:4, :])
        nc.gpsimd.dma_start(w1_t[:, 4:8, :], w1_d[:, 4:8, :])

        with ExitStack() as ph1i:
            strips = ph1i.enter_context(tc.tile_pool(name="strips", bufs=8))
            tpool = ph1i.enter_context(tc.tile_pool(name="tpool", bufs=3))
            xin = ph1i.enter_context(tc.tile_pool(name="xin", bufs=3))
            rpre = ph1i.enter_context(tc.tile_pool(name="rpre", bufs=3))
            stat = ph1i.enter_context(tc.tile_pool(name="stat", bufs=6))
            rtp = ph1i.enter_context(tc.tile_pool(name="rtp", bufs=3))
            f2 = ph1i.enter_context(tc.tile_pool(name="f2", bufs=2))
            ostage = ph1i.enter_context(tc.tile_pool(name="ostage", bufs=2))
            stat2 = ph1i.enter_context(tc.tile_pool(name="stat2", bufs=6))
            ps_s = ph1i.enter_context(
                tc.tile_pool(name="ps_s", bufs=2, space="PSUM"))
            ps_a = ph1i.enter_context(
                tc.tile_pool(name="ps_a", bufs=3, space="PSUM"))
            ps_t = ph1i.enter_context(
                tc.tile_pool(name="ps_t", bufs=1, space="PSUM"))
            ps_f = ph1i.enter_context(
                tc.tile_pool(name="ps_f", bufs=2, space="PSUM"))

            all_halves = {}

            def emit_scores(sb, jlist):
                """z/2 matmul + tanh + per-row centering -> fp8 strips."""
                isl = slice(sb * SB, (sb + 1) * SB)
                if sb not in all_halves:
                    all_halves[sb] = [
                        strips.tile([128, 4, SB], F8E4, tag="strip",
                                    name=f"strip_{sb}_{q}")
                        for q in range(4)]
                halves = all_halves[sb]
                for j in jlist:
                    pst = ps_s.tile([128, SB], F32, tag="s")
                    nc.tensor.matmul(
                        pst, tile_q[:, j * 128:(j + 1) * 128],
                        tile_k[:, isl], start=True, stop=True)
                    tt = tpool.tile([128, SB], F32, tag="tt",
                                    name=f"tt{sb}_{j % 3}")
                    nc.scalar.activation(tt, pst, ACTF.Tanh)
                    nc.vector.tensor_tensor(
                        halves[j // 4][:, j % 4, :], tt, mB[:, isl],
                        op=AluOpType.subtract)

            rT = [None] * NC

            def stage_tr(c):
                rt_ = rtp.tile([128, 8, 128], BF16, tag="rT")
                for h in range(2):
                    ptr = ps_t.tile([128, 512], BF16, tag="tr",
                                    name=f"ptr{h}")
                    for k in range(4):
                        kk = h * 4 + k
                        nc.tensor.transpose(
                            ptr[:, k * 128:(k + 1) * 128],
                            res_b[:, c, kk * 128:(kk + 1) * 128], iden_t)
                    nc.scalar.activation(rt_[:, h * 4:(h + 1) * 4, :], ptr,
                                         ACTF.Copy)
                rT[c] = rt_

            def stage_ff(c):
                pf = {dt2: ps_f.tile([128, 512], F32, tag="f", name=f"pf{_d}")
                      for dt2, _d in ((d_, d_) for d_ in range(ND2))}
                for k in range(8):
                    for dt2 in range(ND2):   # pair shares stationary
                        dsl = slice(dt2 * 512, (dt2 + 1) * 512)
                        nc.tensor.matmul(pf[dt2], rT[c][:, k, :],
                                         w1_t[:, k, dsl],
                                         start=(k == 0),
                                         stop=(k == 7 and not have_b1))
                rT[c] = None
                if have_b1:
                    for dt2 in range(ND2):
                        dsl = slice(dt2 * 512, (dt2 + 1) * 512)
                        nc.tensor.matmul(pf[dt2], ones_t[0:1, 0:128],
                                         b1_t[:, dsl], start=False, stop=True)
                r2 = f2.tile([128, D], F32, tag="r2")
                for dt2 in range(ND2):
                    dsl = slice(dt2 * 512, (dt2 + 1) * 512)
                    nc.vector.scalar_tensor_tensor(
                        out=r2[:, dsl], in0=pf[dt2], scalar=zero_t,
                        in1=res_b[:, c, dsl],
                        op0=AluOpType.max, op1=AluOpType.add)
                st_t = stat2.tile([128, 2, 6], F32, tag="bst2")
                for g in range(2):
                    nc.vector.bn_stats(st_t[:, g, :],
                                       r2[:, g * 512:(g + 1) * 512])
                mv = stat2.tile([128, 2], F32, tag="mv2")
                nc.vector.bn_aggr(mv, st_t)
                rstd = emit_rstd_fast(stat2, mv[:, 1:2], f"r2_{c}")
                o_t = ostage.tile([128, D], F32, tag="o")
                nmr = stat2.tile([128, 1], F32, tag="nmr2", name=f"nm2_{c}")
                nc.vector.tensor_scalar(
                    out=nmr, in0=mv[:, 0:1], scalar1=rstd,
                    scalar2=negone_t, op0=AluOpType.mult,
                    op1=AluOpType.mult)
                nc.scalar.activation(o_t, r2, ACTF.Identity, bias=nmr,
                                     scale=rstd)
                nc.sync.dma_start(out_d[c * 128:(c + 1) * 128, :], o_t)

            x_ts = {}

            def fetch_x(c):
                x_t = xin.tile([128, D], F32, tag="x", name=f"x{c % 3}")
                nc.gpsimd.dma_start(x_t, x_d[c * 128:(c + 1) * 128, :])
                x_ts[c] = x_t

            emit_scores(0, range(NC))
            fetch_x(0)
            for c in range(NC):
                sb, l = c // 4, c % 4
                # produce next superblock's strips while attention runs
                if sb + 1 < NSB:
                    emit_scores(sb + 1, range(l * 4, (l + 1) * 4))
                if c + 1 < NC:
                    fetch_x(c + 1)
                halves = all_halves[sb]
                x_t = x_ts.pop(c)
                rp = rpre.tile([128, D], F32, tag="rp", name=f"rp{c % 3}")
                csl = slice(c * 128, (c + 1) * 128)
                for dt2 in range(ND2):
                    dsl = slice(dt2 * 512, (dt2 + 1) * 512)
                    pa = ps_a.tile([128, 512], F32, tag="a",
                                   name=f"pa{c % 2}_{dt2}")
                    # rank-1 seed: (1+m_i) * colsum(vh)[d]
                    nc.tensor.matmul(pa, onepm[0:1, csl],
                                     csrow[0:1, dsl], start=True, stop=False)
                    for jp in range(8):      # DoubleRow over j-chunk pairs
                        q, m = jp // 2, (jp % 2) * 2
                        nc.tensor.matmul(
                            pa, halves[q][:, m:m + 2, l * 128:(l + 1) * 128],
                            vh_sb[:, 2 * jp:2 * jp + 2, dsl],
                            start=False, stop=(jp == 7),
                            perf_mode=DRMODE)
                    nc.vector.tensor_tensor(
                        rp[:, dsl], pa, x_t[:, dsl], op=AluOpType.add)
                if l == 3:
                    all_halves.pop(sb)
                st_t = stat.tile([128, 2, 6], F32, tag="bst",
                                 name=f"bst{c % 3}")
                for g in range(2):
                    nc.vector.bn_stats(st_t[:, g, :],
                                       rp[:, g * 512:(g + 1) * 512])
                mv = stat.tile([128, 2], F32, tag="mv", name=f"mv{c % 3}")
                nc.vector.bn_aggr(mv, st_t)
                rstd = emit_rstd_fast(stat, mv[:, 1:2], f"r1_{c}")
                if have_gb:
                    t1 = rpre.tile([128, D], F32, tag="t1", name=f"t1{c % 3}")
                    nc.vector.scalar_tensor_tensor(
                        out=t1, in0=rp, scalar=mv[:, 0:1],
                        in1=g1_b,
                        op0=AluOpType.subtract, op1=AluOpType.mult)
                    nc.vector.scalar_tensor_tensor(
                        out=res_b[:, c, :], in0=t1, scalar=rstd,
                        in1=be1_b,
                        op0=AluOpType.mult, op1=AluOpType.add)
                else:
                    nc.vector.tensor_scalar(
                        out=res_b[:, c, :], in0=rp, scalar1=mv[:, 0:1],
                        scalar2=rstd,
                        op0=AluOpType.subtract,
                        op1=AluOpType.mult)
                # interleave FF pipeline for older chunks
                if c >= 1:
                    stage_tr(c - 1)
                if c >= 3:
                    stage_ff(c - 3)
            stage_tr(NC - 1)
            for c in range(NC - 3, NC):
                stage_ff(c)

    nc.finalize()
    return nc


_PROGRAM_CACHE = {}


def _get_program(flags):
    if flags not in _PROGRAM_CACHE:
        _PROGRAM_CACHE[flags] = build_program(flags)
    return _PROGRAM_CACHE[flags]


def kernel(x, Wk, bk, Wq, bq, Wv, bv, Wb, bb, W1, b1, g1, be1):
    import math
    scale = 1.0 / math.sqrt(K)
    flags = (
        bool(np.any(bk) or np.any(bq)),
        bool(np.any(bb)),
        bool(np.any(bv)),
        bool(np.any(b1)),
        bool(np.any(g1 != 1.0) or np.any(be1)),
    )
    nc = _get_program(flags)

    def _prep(w):
        # [D, M] -> [128, 8, M] matching SBUF [partition, chunk, free], bf16
        return np.ascontiguousarray(
            w.astype(np.float32).reshape(8, 128, -1).transpose(1, 0, 2)
        ).astype(ml_dtypes.bfloat16)

    # z/2 trick: keys side gets scale/2, bias side gets 1/2
    wkq = _prep(np.concatenate([Wk * (scale * 0.5), Wq], axis=1))
    wb = _prep(np.asarray(Wb) * 0.5)
    wv = _prep(np.asarray(Wv) * 0.5)
    w1 = _prep(np.asarray(W1))
    onesrow = np.ones((1, S), dtype=ml_dtypes.bfloat16)
    iden = np.eye(128, dtype=ml_dtypes.bfloat16)
    bkq = np.concatenate([np.asarray(bk) * (scale * 0.5), np.asarray(bq)]
                         )[None, :].astype(ml_dtypes.bfloat16)
    bbr = (np.asarray(bb, dtype=np.float32) * 0.5).reshape(1, 1).astype(
        ml_dtypes.bfloat16)
    bvsr = (np.asarray(bv, dtype=np.float32) * 0.5)[None, :].astype(
        ml_dtypes.bfloat16)
    bvcsr = (np.asarray(bv, dtype=np.float32) * (0.5 * S))[None, :].astype(
        ml_dtypes.bfloat16)
    b1r = np.asarray(b1, dtype=np.float32)[None, :].astype(ml_dtypes.bfloat16)
    g1r = np.asarray(g1, dtype=np.float32)[None, :]
    be1r = np.asarray(be1, dtype=np.float32)[None, :]

    xb = np.ascontiguousarray(np.transpose(x, (0, 2, 1)))        # [B,D,S]
    xb = np.ascontiguousarray(
        xb.reshape(B, 8, 128, S).transpose(0, 2, 1, 3)            # [B,128,8,S]
    ).astype(ml_dtypes.bfloat16)

    in_maps = []
    for b in range(B):
        in_maps.append(dict(
            xb=xb[b], x=np.ascontiguousarray(x[b], dtype=np.float32),
            wkq=wkq, wb=wb, wv=wv, w1=w1, onesrow=onesrow, iden=iden,
            g1=g1r, be1=be1r, bkq=bkq, bb=bbr, bvs=bvsr, bvcs=bvcsr,
            b1=b1r))

    res = run_bass_kernel_spmd(nc, in_maps, list(range(NCORES)), trace=False)
    out = np.stack([res.results[b]["out"] for b in range(B)], axis=0)
    return out.astype(np.float32)
